# revision 1
# baseline (speedup 1.0000x reference)
"""Trainium2 Bass kernel for a pre-RMSNorm attention+FFN transformer block.

Problem: x (2, 1024, 4096) fp32, channel-major (B, C, T).
  h = x^T; h += Attn(RMSNorm(h)); h += FFN(RMSNorm(h)); return h^T.

Sharding: 8 cores = 2 batches x 4 query-token chunks of 1024.  Each core
computes K/V for its batch's full 4096 tokens (redundantly within the
4-core group -- avoids all collectives), attention + Wo + FFN for its own
1024-token chunk.  Host slices inputs and concatenates the 8 output chunks.

All matmuls run in bf16 (fp32 PSUM accumulation); the residual path stays
fp32.  Everything is kept channel-major so no transposes are needed:
x[b] is already (C, T), weights load in natural layout.
"""

import numpy as np
import ml_dtypes

import concourse.bass as bass
import concourse.mybir as mybir
import concourse.tile as tile
from concourse import bacc
from concourse.bass_utils import run_bass_kernel_spmd

F32 = mybir.dt.float32
BF16 = mybir.dt.bfloat16
AF = mybir.ActivationFunctionType

B = 2
C = 1024
T = 4096
TQ = 1024          # query-token chunk per core
H = 4
DH = 256
FF = 1536
P = 128
NT = 512           # moving-operand / PSUM tile width
CT = C // P        # 8 channel tiles
TT = T // NT       # 8 full-T token tiles
TQT = TQ // NT     # 2 chunk token tiles
DB = C // P        # 8 output-channel blocks for q/k/v/o
FFB = FF // P      # 12 ff blocks
TJ = T // P        # 32 key-token blocks

_CACHE = {}


def _rmsnorm_tiles(nc, pool, psum_pool, ones_t, eps_t, xt, aT, tags):
    """xt: [P, CT*NT] fp32 (channel-major, CT c-tiles of one NT-token tile).
    Writes aT [P, CT*NT] bf16 = xt * rsqrt(mean_c(xt^2) + eps)."""
    sqtag, sstag, sqttag, rntag = tags
    ss = psum_pool.tile([P, NT], F32, tag=sstag, name="ss_" + sstag)
    for ci in range(CT):
        sq = pool.tile([P, NT], BF16, tag=sqtag, bufs=2, name="sq_" + sqtag)
        nc.scalar.activation(sq[:], xt[:, ci * NT:(ci + 1) * NT], AF.Square)
        nc.tensor.matmul(ss[:], ones_t[:], sq[:], start=(ci == 0), stop=(ci == CT - 1))
    sqt = pool.tile([P, NT], F32, tag=sqttag, bufs=2, name="sqt_" + sqttag)
    nc.scalar.activation(sqt[:], ss[:], AF.Sqrt, scale=1.0 / C, bias=eps_t[:])
    rn = pool.tile([P, NT], F32, tag=rntag, bufs=2, name="rn_" + rntag)
    nc.vector.reciprocal(rn[:], sqt[:])
    for ci in range(CT):
        nc.vector.tensor_mul(aT[:, ci * NT:(ci + 1) * NT],
                             xt[:, ci * NT:(ci + 1) * NT], rn[:])


def _build():
    nc = bacc.Bacc()
    xqb = nc.dram_tensor("xqb", [C, TQ], BF16, kind="ExternalInput")   # bf16 chunk
    xq = nc.dram_tensor("xq", [C, TQ], F32, kind="ExternalInput")      # fp32 chunk (residual)
    wq = nc.dram_tensor("wq", [C, C], BF16, kind="ExternalInput")
    wk = nc.dram_tensor("wk", [C, C], BF16, kind="ExternalInput")
    wv = nc.dram_tensor("wv", [C, C], BF16, kind="ExternalInput")
    wo = nc.dram_tensor("wo", [C, C], BF16, kind="ExternalInput")
    w1 = nc.dram_tensor("w1", [C, FF], BF16, kind="ExternalInput")
    w2 = nc.dram_tensor("w2", [FF, C], BF16, kind="ExternalInput")
    out = nc.dram_tensor("out", [C, TQ], F32, kind="ExternalOutput")

    RG = [[0, 1, 2, 3], [4, 5, 6, 7]]

    with tile.TileContext(nc) as tc:
        cpool_cm = tc.tile_pool(name="const", bufs=1)
        cpool = cpool_cm.__enter__()
        ones_t = cpool.tile([P, P], BF16, tag="ones", name="ones_t")
        nc.vector.memset(ones_t[:], 1.0)
        eps_t = cpool.tile([P, 1], F32, tag="eps", name="eps_t")
        nc.vector.memset(eps_t[:], 1e-8)

        dram_cm = tc.tile_pool(name="dram", bufs=1, space="DRAM")
        dp = dram_cm.__enter__()
        kl_d = dp.tile([DB * P, TQ], BF16, tag="kl_d", name="kl_d")
        vl_d = dp.tile([(TQ // P) * P, C], BF16, tag="vl_d", name="vl_d")
        kg_d = dp.tile([4 * DB * P, TQ], BF16, tag="kg_d", name="kg_d")
        vg_d = dp.tile([4 * (TQ // P) * P, C], BF16, tag="vg_d", name="vg_d")

        # right-side stack: qT (B..D), then B's weights+activations (B only)
        qo_cm = tc.tile_pool(name="qopool", bufs=1, side="right")
        qop = qo_cm.__enter__()
        qT = qop.tile([P, DB * TQ], BF16, tag="qT", name="qT")          # 16KB
        oT = qT  # o^T reuses q^T's buffer: each (head, ti) slice is dead after its scores

        pbA_cm = tc.tile_pool(name="pbA", bufs=1, side="right")
        pbA = pbA_cm.__enter__()
        wq_sb = pbA.tile([P, CT * C], BF16, tag="wq_sb", name="wq_sb")
        wk_sb = pbA.tile([P, CT * C], BF16, tag="wk_sb", name="wk_sb")
        wv_sb = pbA.tile([P, CT * C], BF16, tag="wv_sb", name="wv_sb")
        for ci in range(CT):
            nc.gpsimd.dma_start(wk_sb[:, ci * C:(ci + 1) * C], wk[ci * P:(ci + 1) * P, :])
            nc.gpsimd.dma_start(wv_sb[:, ci * C:(ci + 1) * C], wv[ci * P:(ci + 1) * P, :])
            nc.gpsimd.dma_start(wq_sb[:, ci * C:(ci + 1) * C], wq[ci * P:(ci + 1) * P, :])

        pbps_cm = tc.tile_pool(name="pb_ps", bufs=2, space="PSUM")
        pbps = pbps_cm.__enter__()

        # ---- chunk rmsnorm (norm transients in a short-lived left pool) ----
        aTb_list = []
        pbN_cm = tc.tile_pool(name="pbN", bufs=1)
        pbN = pbN_cm.__enter__()
        for t2 in range(TQT):
            xt = pbN.tile([P, CT * NT], BF16, tag="xqt", bufs=2, name="xqt")
            for ci in range(CT):
                nc.sync.dma_start(xt[:, ci * NT:(ci + 1) * NT],
                                  xqb[ci * P:(ci + 1) * P, t2 * NT:(t2 + 1) * NT])
            ss = pbps.tile([P, NT], F32, tag="ssb", name="ssb")
            for ci in range(CT):
                sq = pbN.tile([P, NT], BF16, tag="sqb", bufs=2, name="sqb")
                nc.scalar.activation(sq[:], xt[:, ci * NT:(ci + 1) * NT], AF.Square)
                nc.tensor.matmul(ss[:], ones_t[:], sq[:], start=(ci == 0), stop=(ci == CT - 1))
            sqt = pbN.tile([P, NT], F32, tag="sqtb", bufs=2, name="sqtb")
            nc.scalar.activation(sqt[:], ss[:], AF.Sqrt, scale=1.0 / C, bias=eps_t[:])
            rn = pbN.tile([P, NT], F32, tag="rnb", bufs=2, name="rnb")
            nc.vector.reciprocal(rn[:], sqt[:])
            aT = pbA.tile([P, CT * NT], BF16, tag="aTb", bufs=2, name="aTb")
            for ci in range(CT):
                nc.vector.tensor_mul(aT[:, ci * NT:(ci + 1) * NT],
                                     xt[:, ci * NT:(ci + 1) * NT], rn[:])
            aTb_list.append(aT)
        pbN_cm.__exit__(None, None, None)

        # ---- K chunk -> kl_d -> AllGather(k) ----
        for db in range(DB):
            for t2 in range(TQT):
                pk = pbps.tile([P, NT], F32, tag="pk", name="pk")
                for ci in range(CT):
                    nc.tensor.matmul(pk[:],
                                     wk_sb[:, ci * C + db * P: ci * C + (db + 1) * P],
                                     aTb_list[t2][:, ci * NT:(ci + 1) * NT],
                                     start=(ci == 0), stop=(ci == CT - 1))
                st = pbA.tile([P, NT], BF16, tag="stg", bufs=3, name="stg")
                nc.vector.tensor_copy(st[:], pk[:])
                nc.sync.dma_start(kl_d[db * P:(db + 1) * P, t2 * NT:(t2 + 1) * NT], st[:])
        nc.gpsimd.collective_compute(
            "AllGather", mybir.AluOpType.bypass, replica_groups=RG,
            ins=[kl_d[:, :]], outs=[kg_d[:, :]])

        # k^T full buffer: loads overlap the V/Q matmuls below
        kT_cm = tc.tile_pool(name="kTpool", bufs=1)
        kTp = kT_cm.__enter__()
        kT = kTp.tile([P, DB * T], BF16, tag="kT", name="kT")          # 64KB/part
        for db in range(DB):
            for r in range(4):
                nc.sync.dma_start(
                    kT[:, db * T + r * TQ: db * T + (r + 1) * TQ],
                    kg_d[r * DB * P + db * P: r * DB * P + (db + 1) * P, :])

        # ---- V chunk -> vl_d -> AllGather(v) ----
        for t2 in range(TQT):
            for tb in range(NT // P):
                jl = t2 * (NT // P) + tb
                for hf in range(2):
                    pv = pbps.tile([P, NT], F32, tag="pv", name="pv")
                    for ci in range(CT):
                        nc.tensor.matmul(pv[:],
                                         aTb_list[t2][:, ci * NT + tb * P: ci * NT + (tb + 1) * P],
                                         wv_sb[:, ci * C + hf * NT: ci * C + (hf + 1) * NT],
                                         start=(ci == 0), stop=(ci == CT - 1))
                    st = pbA.tile([P, NT], BF16, tag="stg", bufs=3, name="stg2")
                    nc.vector.tensor_copy(st[:], pv[:])
                    nc.sync.dma_start(vl_d[jl * P:(jl + 1) * P, hf * NT:(hf + 1) * NT], st[:])
        nc.gpsimd.collective_compute(
            "AllGather", mybir.AluOpType.bypass, replica_groups=RG,
            ins=[vl_d[:, :]], outs=[vg_d[:, :]])

        # ---- Q (overlaps the k loads / v gather) ----
        for t2 in range(TQT):
            for db in range(DB):
                pq = pbps.tile([P, NT], F32, tag="pq", name="pq")
                for ci in range(CT):
                    nc.tensor.matmul(pq[:],
                                     wq_sb[:, ci * C + db * P: ci * C + (db + 1) * P],
                                     aTb_list[t2][:, ci * NT:(ci + 1) * NT],
                                     start=(ci == 0), stop=(ci == CT - 1))
                nc.vector.tensor_copy(qT[:, db * TQ + t2 * NT: db * TQ + (t2 + 1) * NT], pq[:])
        pbps_cm.__exit__(None, None, None)
        pbA_cm.__exit__(None, None, None)

        # v full buffer, loaded from the gathered DRAM (overlaps attention start)
        vB_cm = tc.tile_pool(name="vBpool", bufs=1)
        vBp = vB_cm.__enter__()
        vB = vBp.tile([P, TJ * C], BF16, tag="vB", name="vB")          # 64KB/part
        for r in range(4):
            for jl in range(TQ // P):
                j = r * (TQ // P) + jl
                nc.sync.dma_start(
                    vB[:, j * C:(j + 1) * C],
                    vg_d[r * TQ + jl * P: r * TQ + (jl + 1) * P, :])

        # ---------------- phase C: attention ----------------
        wo_cm = tc.tile_pool(name="wopool", bufs=1, side="right")
        wop = wo_cm.__enter__()
        wo_sb = wop.tile([P, CT * C], BF16, tag="wo_sb", name="wo_sb")
        for ci in range(CT):
            nc.gpsimd.dma_start(wo_sb[:, ci * C:(ci + 1) * C], wo[ci * P:(ci + 1) * P, :])
        pc_cm = tc.tile_pool(name="pc", bufs=1)
        pcp = pc_cm.__enter__()
        pss_cm = tc.tile_pool(name="ps_s", bufs=2, space="PSUM")
        pss = pss_cm.__enter__()
        pso_cm = tc.tile_pool(name="ps_o", bufs=2, space="PSUM")
        pso = pso_cm.__enter__()
        NHALF = 16
        for h in range(H):
            for ti in range(TQT):
                po0 = pso.tile([P, NT], F32, tag="po0", name="po0")
                po1 = pso.tile([P, NT], F32, tag="po1", name="po1")
                pr = pso.tile([P, NT], F32, tag="pr", bufs=1, name="pr")
                for half in range(2):
                    et = pcp.tile([P, NHALF * NT], BF16, tag="exp", bufs=2, name="et")
                    for jj in range(NHALF):
                        tj = half * NHALF + jj
                        psc = pss.tile([P, NT], F32, tag="s", bufs=3, name="psc")
                        for dd in range(2):
                            db = 2 * h + dd
                            nc.tensor.matmul(psc[:],
                                             kT[:, db * T + tj * P: db * T + (tj + 1) * P],
                                             qT[:, db * TQ + ti * NT: db * TQ + (ti + 1) * NT],
                                             start=(dd == 0), stop=(dd == 1))
                        nc.scalar.activation(et[:, jj * NT:(jj + 1) * NT], psc[:],
                                             AF.Exp, scale=float(DH) ** -0.5)
                    for jj in range(NHALF):
                        tj = half * NHALF + jj
                        st_, sp_ = (tj == 0), (tj == TJ - 1)
                        e_sl = et[:, jj * NT:(jj + 1) * NT]
                        nc.tensor.matmul(po0[:], vB[:, tj * C + h * DH: tj * C + h * DH + P],
                                         e_sl, start=st_, stop=sp_, skip_group_check=True)
                        nc.tensor.matmul(po1[:], vB[:, tj * C + h * DH + P: tj * C + (h + 1) * DH],
                                         e_sl, start=st_, stop=sp_, skip_group_check=True)
                    # depth-4 bf16 pre-reduction on DVE: one rowsum matmul per 16 tiles
                    oc = []
                    for jj8 in range(0, NHALF, 8):
                        qs = []
                        for q in range(2):
                            jb = jj8 + 4 * q
                            pe_a = pcp.tile([P, NT], BF16, tag="pe_a", bufs=2, name="pe_a")
                            nc.vector.tensor_add(pe_a[:], et[:, jb * NT:(jb + 1) * NT],
                                                 et[:, (jb + 1) * NT:(jb + 2) * NT])
                            pe_b = pcp.tile([P, NT], BF16, tag="pe_b", bufs=2, name="pe_b")
                            nc.vector.tensor_add(pe_b[:], et[:, (jb + 2) * NT:(jb + 3) * NT],
                                                 et[:, (jb + 3) * NT:(jb + 4) * NT])
                            pe_q = pcp.tile([P, NT], BF16, tag="pe_q", bufs=2, name="pe_q")
                            nc.vector.tensor_add(pe_q[:], pe_a[:], pe_b[:])
                            qs.append(pe_q)
                        pe_o = pcp.tile([P, NT], BF16, tag="pe_o", bufs=2, name="pe_o")
                        nc.vector.tensor_add(pe_o[:], qs[0][:], qs[1][:])
                        oc.append(pe_o)
                    pe_t = pcp.tile([P, NT], BF16, tag="pe_t", bufs=2, name="pe_t")
                    nc.vector.tensor_add(pe_t[:], oc[0][:], oc[1][:])
                    st_, sp_ = (half == 0), (half == 1)
                    nc.tensor.matmul(pr[:], ones_t[:], pe_t[:],
                                     start=st_, stop=sp_, skip_group_check=True)
                rec = pcp.tile([P, NT], F32, tag="rec", bufs=2, name="rec")
                nc.vector.reciprocal(rec[:], pr[:])
                nc.vector.tensor_mul(oT[:, (2 * h) * TQ + ti * NT:(2 * h) * TQ + (ti + 1) * NT],
                                     po0[:], rec[:])
                nc.vector.tensor_mul(oT[:, (2 * h + 1) * TQ + ti * NT:(2 * h + 1) * TQ + (ti + 1) * NT],
                                     po1[:], rec[:])
        pso_cm.__exit__(None, None, None)
        pss_cm.__exit__(None, None, None)
        pc_cm.__exit__(None, None, None)
        vB_cm.__exit__(None, None, None)
        kT_cm.__exit__(None, None, None)

        # ---------------- phase D: Wo + residual ----------------
        h_cm = tc.tile_pool(name="hpool", bufs=1)
        hp = h_cm.__enter__()
        hB = hp.tile([P, CT * TQ], F32, tag="hB", name="hB")            # 32KB
        pd_cm = tc.tile_pool(name="pd", bufs=1)
        pdp = pd_cm.__enter__()
        pdps_cm = tc.tile_pool(name="pd_ps", bufs=2, space="PSUM")
        pdps = pdps_cm.__enter__()
        xqD = pdp.tile([P, CT * TQ], F32, tag="xqD", name="xqD")        # 32KB
        for ci in range(CT):
            nc.sync.dma_start(xqD[:, ci * TQ:(ci + 1) * TQ], xq[ci * P:(ci + 1) * P, :])
        for cb in range(CT):
            for t2 in range(TQT):
                ph = pdps.tile([P, NT], F32, tag="ph", bufs=4, name="ph")
                for cp_ in range(CT):
                    nc.tensor.matmul(ph[:],
                                     wo_sb[:, cp_ * C + cb * P: cp_ * C + (cb + 1) * P],
                                     oT[:, cp_ * TQ + t2 * NT: cp_ * TQ + (t2 + 1) * NT],
                                     start=(cp_ == 0), stop=(cp_ == CT - 1))
                nc.vector.tensor_add(hB[:, cb * TQ + t2 * NT: cb * TQ + (t2 + 1) * NT],
                                     ph[:], xqD[:, cb * TQ + t2 * NT: cb * TQ + (t2 + 1) * NT])
        pdps_cm.__exit__(None, None, None)
        pd_cm.__exit__(None, None, None)
        wo_cm.__exit__(None, None, None)
        qo_cm.__exit__(None, None, None)

        # ---------------- phase E: FFN ----------------
        pe_cm = tc.tile_pool(name="pe", bufs=1)
        pep = pe_cm.__enter__()
        peps_cm = tc.tile_pool(name="pe_ps", bufs=2, space="PSUM")
        peps = peps_cm.__enter__()
        w1_sb = pep.tile([P, CT * FF], BF16, tag="w1_sb", name="w1_sb")   # 24KB
        for ci in range(CT):
            nc.gpsimd.dma_start(w1_sb[:, ci * FF:(ci + 1) * FF], w1[ci * P:(ci + 1) * P, :])
        w2_sb = pep.tile([P, FFB * C], BF16, tag="w2_sb", name="w2_sb")   # 24KB
        for fi in range(FFB):
            nc.gpsimd.dma_start(w2_sb[:, fi * C:(fi + 1) * C], w2[fi * P:(fi + 1) * P, :])
        fB = pep.tile([P, CT * TQ], BF16, tag="fB", name="fB")            # 16KB
        gB = pep.tile([P, FFB * TQ], BF16, tag="gB", name="gB")           # 24KB
        for t2 in range(TQT):
            ss = peps.tile([P, NT], F32, tag="sse", name="sse")
            for ci in range(CT):
                sq = pep.tile([P, NT], BF16, tag="sqe", bufs=2, name="sqe")
                nc.scalar.activation(sq[:], hB[:, ci * TQ + t2 * NT: ci * TQ + (t2 + 1) * NT], AF.Square)
                nc.tensor.matmul(ss[:], ones_t[:], sq[:], start=(ci == 0), stop=(ci == CT - 1))
            sqt = pep.tile([P, NT], F32, tag="sqte", bufs=2, name="sqte")
            nc.scalar.activation(sqt[:], ss[:], AF.Sqrt, scale=1.0 / C, bias=eps_t[:])
            rn = pep.tile([P, NT], F32, tag="rne", bufs=2, name="rne")
            nc.vector.reciprocal(rn[:], sqt[:])
            for ci in range(CT):
                nc.vector.tensor_mul(fB[:, ci * TQ + t2 * NT: ci * TQ + (t2 + 1) * NT],
                                     hB[:, ci * TQ + t2 * NT: ci * TQ + (t2 + 1) * NT], rn[:])
        for fb in range(FFB):
            for t2 in range(TQT):
                pu = peps.tile([P, NT], F32, tag="pu", bufs=3, name="pu")
                for ci in range(CT):
                    nc.tensor.matmul(pu[:],
                                     w1_sb[:, ci * FF + fb * P: ci * FF + (fb + 1) * P],
                                     fB[:, ci * TQ + t2 * NT: ci * TQ + (t2 + 1) * NT],
                                     start=(ci == 0), stop=(ci == CT - 1))
                nc.scalar.activation(gB[:, fb * TQ + t2 * NT: fb * TQ + (t2 + 1) * NT],
                                     pu[:], AF.Gelu)
        for cb in range(CT):
            for t2 in range(TQT):
                py = peps.tile([P, NT], F32, tag="py", bufs=3, name="py")
                for fb in range(FFB):
                    nc.tensor.matmul(py[:],
                                     w2_sb[:, fb * C + cb * P: fb * C + (cb + 1) * P],
                                     gB[:, fb * TQ + t2 * NT: fb * TQ + (t2 + 1) * NT],
                                     start=(fb == 0), stop=(fb == FFB - 1))
                yt = pep.tile([P, NT], F32, tag="yt", bufs=3, name="yt")
                nc.vector.tensor_add(yt[:], py[:], hB[:, cb * TQ + t2 * NT: cb * TQ + (t2 + 1) * NT])
                nc.sync.dma_start(out[cb * P:(cb + 1) * P, t2 * NT:(t2 + 1) * NT], yt[:])
        peps_cm.__exit__(None, None, None)
        pe_cm.__exit__(None, None, None)
        h_cm.__exit__(None, None, None)
        dram_cm.__exit__(None, None, None)
        cpool_cm.__exit__(None, None, None)

        sched_state, snap = tc.schedule_and_allocate()
        _CACHE["predicted_ns"] = snap.time if snap is not None else None
        try:
            _CACHE["dispatch_ns"] = sched_state.get_inst_dispatch_ns()
        except Exception:
            _CACHE["dispatch_ns"] = None

    nc.finalize()
    return nc


def get_nc():
    if "nc" not in _CACHE:
        _CACHE["nc"] = _build()
    return _CACHE["nc"]


def _prep_inputs(inputs):
    bf = ml_dtypes.bfloat16
    x = np.asarray(inputs["x"], dtype=np.float32)
    g_attn = np.asarray(inputs["g_attn"], dtype=np.float32)
    g_ff = np.asarray(inputs["g_ff"], dtype=np.float32)
    wqb = (g_attn[:, None] * np.asarray(inputs["Wq"], np.float32)).astype(bf)
    wkb = (g_attn[:, None] * np.asarray(inputs["Wk"], np.float32)).astype(bf)
    wvb = (g_attn[:, None] * np.asarray(inputs["Wv"], np.float32)).astype(bf)
    wob = np.asarray(inputs["Wo"], np.float32).astype(bf)
    w1b = (g_ff[:, None] * np.asarray(inputs["W1"], np.float32)).astype(bf)
    w2b = np.asarray(inputs["W2"], np.float32).astype(bf)
    xbf = x.astype(bf)
    in_maps = []
    for core in range(8):
        b, cq = divmod(core, 4)
        in_maps.append({
            "xqb": np.ascontiguousarray(xbf[b][:, cq * TQ:(cq + 1) * TQ]),
            "xq": np.ascontiguousarray(x[b][:, cq * TQ:(cq + 1) * TQ]),
            "wq": wqb, "wk": wkb, "wv": wvb, "wo": wob, "w1": w1b, "w2": w2b,
        })
    return in_maps


def run(inputs, **kwargs):
    nc = get_nc()
    in_maps = _prep_inputs(inputs)
    res = run_bass_kernel_spmd(nc, in_maps, core_ids=list(range(8)), **kwargs)
    out = np.empty((B, C, T), np.float32)
    for core in range(8):
        b, cq = divmod(core, 4)
        out[b][:, cq * TQ:(cq + 1) * TQ] = res.results[core]["out"]
    return out, res


def kernel(**inputs) -> np.ndarray:
    out, _ = run(inputs)
    return out



# revision 9
# speedup vs baseline: 1.6417x; 1.6417x over previous
"""Trainium2 Bass kernel for a pre-RMSNorm attention+FFN transformer block.

Problem: x (2, 1024, 4096) fp32, channel-major (B, C, T).
  h = x^T; h += Attn(RMSNorm(h)); h += FFN(RMSNorm(h)); return h^T.

Sharding: 8 cores = 2 batches x 4 query-token chunks of 1024.  Each core
computes K/V for its batch's own 1024-token chunk, AllGathers K/V within
its 4-core batch group, then runs attention + Wo + FFN for its own chunk.

All matmuls run in fp8(e4m3) with DoubleRow perf mode (K=256 per
instruction, 0.5 cycles/row) accumulating in fp32 PSUM.  Weights are
prescaled by 32 on the host to center their distribution in the fp8
normal range; the scale is folded back out in the exp scale (scores),
the gelu scale (W1) and scalar_tensor_tensor residual adds (Wo, W2).
Residual path stays fp32.  Softmax row-sums are computed on the PE with
a DoubleRow ones-matmul over the fp8 exp tiles.
"""

import numpy as np
import ml_dtypes

import concourse.bass as bass
import concourse.mybir as mybir
import concourse.tile as tile
from concourse import bacc
from concourse.bass_utils import run_bass_kernel_spmd

F32 = mybir.dt.float32
BF16 = mybir.dt.bfloat16
F8 = mybir.dt.float8e4
AF = mybir.ActivationFunctionType
PM = mybir.MatmulPerfMode.DoubleRow
MUL = mybir.AluOpType.mult
ADD = mybir.AluOpType.add

B = 2
C = 1024
T = 4096
TQ = 1024          # query-token chunk per core
H = 4
DH = 256
FF = 1536
P = 128
NT = 512
CT = C // P        # 8 channel tiles
DB = C // P        # 8 output-channel blocks
FFB = FF // P      # 12 ff blocks
TJ = T // P        # 32 key-token blocks
TQT = TQ // NT     # 2 chunk token tiles
KP = CT // 2       # 4 DoubleRow k-pairs for a C contraction
FKP = FFB // 2     # 6 DoubleRow k-pairs for the FF contraction
SC = 32.0          # host-side weight prescale (fp8 range centering)

_CACHE = {}


def _build():
    nc = bacc.Bacc()
    xb = nc.dram_tensor("xb", [C, TQ], BF16, kind="ExternalInput")    # bf16 chunk
    xq = nc.dram_tensor("xq", [C, TQ], F32, kind="ExternalInput")     # fp32 residual
    wq = nc.dram_tensor("wq", [C, C], F8, kind="ExternalInput")
    wk = nc.dram_tensor("wk", [C, C], F8, kind="ExternalInput")
    wv = nc.dram_tensor("wv", [C, C], F8, kind="ExternalInput")
    wo = nc.dram_tensor("wo", [C, C], F8, kind="ExternalInput")
    w1 = nc.dram_tensor("w1", [C, FF], F8, kind="ExternalInput")
    w2 = nc.dram_tensor("w2", [FF, C], F8, kind="ExternalInput")
    out = nc.dram_tensor("out", [C, TQ], F32, kind="ExternalOutput")

    RG = [[0, 1, 2, 3], [4, 5, 6, 7]]

    def dr3(ap2d, p=P):
        # [A*P, F] dram AP -> [P, A, F] (partition-major blocks of 128 rows)
        return ap2d.rearrange("(a p) f -> p a f", p=p)

    with tile.TileContext(nc) as tc:
        cp_cm = tc.tile_pool(name="const", bufs=1)
        cp = cp_cm.__enter__()
        ones_t = cp.tile([P, P], BF16, tag="ones", name="ones_t")
        nc.vector.memset(ones_t[:], 1.0)
        ones8 = cp.tile([P, 2, P], F8, tag="ones8", name="ones8")
        nc.vector.memset(ones8[:], 1.0)
        eps_t = cp.tile([P, 1], F32, tag="eps", name="eps_t")
        nc.vector.memset(eps_t[:], 1e-8)

        dram_cm = tc.tile_pool(name="dram", bufs=1, space="DRAM")
        dp = dram_cm.__enter__()
        kl_d = dp.tile([C, TQ], F8, tag="kl_d", name="kl_d")
        vl_d = dp.tile([TQ, C], F8, tag="vl_d", name="vl_d")
        kg_d = dp.tile([4 * C, TQ], F8, tag="kg_d", name="kg_d")
        vg_d = dp.tile([4 * TQ, C], F8, tag="vg_d", name="vg_d")

        # ---- weights (fp8, one batched DMA per tensor) ----
        wB_cm = tc.tile_pool(name="wB", bufs=1, side="right")
        wB = wB_cm.__enter__()
        wo_sb = wB.tile([P, CT, C], F8, tag="wo_sb", name="wo_sb")
        w1_sb = wB.tile([P, CT, FF], F8, tag="w1_sb", name="w1_sb")
        w2_sb = wB.tile([P, FFB, C], F8, tag="w2_sb", name="w2_sb")
        wA_cm = tc.tile_pool(name="wA", bufs=1, side="right")
        wA = wA_cm.__enter__()
        wk_sb = wA.tile([P, CT, C], F8, tag="wk_sb", name="wk_sb")
        wv_sb = wA.tile([P, CT, C], F8, tag="wv_sb", name="wv_sb")
        wq_sb = wA.tile([P, CT, C], F8, tag="wq_sb", name="wq_sb")
        nc.gpsimd.dma_start(wk_sb[:, :, :], dr3(wk[:, :]))
        nc.gpsimd.dma_start(wv_sb[:, :, :], dr3(wv[:, :]))
        nc.gpsimd.dma_start(wq_sb[:, :, :], dr3(wq[:, :]))
        nc.gpsimd.dma_start(wo_sb[:, :, :], dr3(wo[:, :]))
        nc.gpsimd.dma_start(w1_sb[:, :, :], dr3(w1[:, :]))
        nc.gpsimd.dma_start(w2_sb[:, :, :], dr3(w2[:, :]))

        hx_cm = tc.tile_pool(name="hx", bufs=1)
        hxp = hx_cm.__enter__()
        hB = hxp.tile([P, CT, TQ], F32, tag="hB", name="hB")          # 32KB/part
        pe_cm = tc.tile_pool(name="pe", bufs=1)
        pep = pe_cm.__enter__()
        fB = pep.tile([P, CT, TQ], F8, tag="fB", name="fB")            # 8KB/part
        gB = pep.tile([P, FFB, TQ], F8, tag="gB", name="gB")           # 12KB/part
        qo_cm = tc.tile_pool(name="qop", bufs=1)
        qop = qo_cm.__enter__()
        qT = qop.tile([P, DB, TQ], F8, tag="qT", name="qT")           # 8KB/part
        oT = qop.tile([P, DB, TQ], F8, tag="oT", name="oT")           # 8KB/part
        aT_cm = tc.tile_pool(name="aTp", bufs=1, side="right")
        aTp = aT_cm.__enter__()
        aT = aTp.tile([P, CT, TQ], F8, tag="aT", name="aT")           # 8KB/part

        pps_cm = tc.tile_pool(name="pps", bufs=1, space="PSUM")
        pps = pps_cm.__enter__()

        # ---- chunk rmsnorm -> aT fp8 ----
        rms_cm = tc.tile_pool(name="rms1", bufs=1)
        rms = rms_cm.__enter__()
        for t2 in range(TQT):
            xt = rms.tile([P, CT, NT], BF16, tag="xt", bufs=2, name="xt")
            nc.gpsimd.dma_start(xt[:, :, :], dr3(xb[:, t2 * NT:(t2 + 1) * NT]))
            ss = pps.tile([P, NT], F32, tag="pp", bufs=4, name="ss")
            for ci in range(CT):
                sq = rms.tile([P, NT], BF16, tag="sq", bufs=2, name="sq")
                nc.vector.tensor_mul(sq[:], xt[:, ci:ci + 1, :], xt[:, ci:ci + 1, :])
                nc.tensor.matmul(ss[:], ones_t[:], sq[:], start=(ci == 0), stop=(ci == CT - 1))
            sqt = rms.tile([P, NT], F32, tag="sqt", bufs=2, name="sqt")
            nc.scalar.activation(sqt[:], ss[:], AF.Sqrt, scale=1.0 / C, bias=eps_t[:])
            rn = rms.tile([P, NT], F32, tag="rn", bufs=2, name="rn")
            nc.vector.reciprocal(rn[:], sqt[:])
            for ci in range(CT):
                nc.vector.tensor_mul(aT[:, ci:ci + 1, t2 * NT:(t2 + 1) * NT],
                                     xt[:, ci:ci + 1, :], rn[:])
        rms_cm.__exit__(None, None, None)

        stg_cm = tc.tile_pool(name="stg", bufs=1, side="right")
        stg = stg_cm.__enter__()

        # ---- K chunk (DoubleRow fp8) -> kl_d -> AllGather ----
        for db in range(DB):
            for t2 in range(TQT):
                pk = pps.tile([P, NT], F32, tag="pp", bufs=4, name="pk")
                for k in range(KP):
                    nc.tensor.matmul(pk[:],
                                     wk_sb[:, 2 * k:2 * k + 2, db * P:(db + 1) * P],
                                     aT[:, 2 * k:2 * k + 2, t2 * NT:(t2 + 1) * NT],
                                     start=(k == 0), stop=(k == KP - 1), perf_mode=PM)
                st = stg.tile([P, NT], F8, tag="stk", bufs=3, name="stk")
                nc.vector.tensor_copy(st[:], pk[:])
                nc.sync.dma_start(kl_d[db * P:(db + 1) * P, t2 * NT:(t2 + 1) * NT], st[:])
        nc.gpsimd.collective_compute(
            "AllGather", mybir.AluOpType.bypass, replica_groups=RG,
            ins=[kl_d[:, :]], outs=[kg_d[:, :]])

        # kT loads (one batched DMA per db, in scores-consumption order)
        kT_cm = tc.tile_pool(name="kTp", bufs=1)
        kTp = kT_cm.__enter__()
        kT = kTp.tile([P, DB, T], F8, tag="kT", name="kT")            # 32KB/part
        kg4 = kg_d[:, :].rearrange("(r a p) f -> p a r f", p=P, a=DB)
        for db in range(DB):
            nc.sync.dma_start(kT[:, db:db + 1, :], kg4[:, db:db + 1, :, :])

        # ---- V chunk (DoubleRow fp8) -> vl_d -> AllGather ----
        for jl in range(TQ // P):
            for hf in range(2):
                pv = pps.tile([P, NT], F32, tag="pp", bufs=4, name="pv")
                for k in range(KP):
                    nc.tensor.matmul(pv[:],
                                     aT[:, 2 * k:2 * k + 2, jl * P:(jl + 1) * P],
                                     wv_sb[:, 2 * k:2 * k + 2, hf * NT:(hf + 1) * NT],
                                     start=(k == 0), stop=(k == KP - 1), perf_mode=PM)
                st = stg.tile([P, NT], F8, tag="stv", bufs=3, name="stv")
                nc.vector.tensor_copy(st[:], pv[:])
                nc.sync.dma_start(vl_d[jl * P:(jl + 1) * P, hf * NT:(hf + 1) * NT], st[:])
        nc.gpsimd.collective_compute(
            "AllGather", mybir.AluOpType.bypass, replica_groups=RG,
            ins=[vl_d[:, :]], outs=[vg_d[:, :]])

        # ---- Q (DoubleRow fp8) ----
        for t2 in range(TQT):
            for db in range(DB):
                pq = pps.tile([P, NT], F32, tag="pp", bufs=4, name="pq")
                for k in range(KP):
                    nc.tensor.matmul(pq[:],
                                     wq_sb[:, 2 * k:2 * k + 2, db * P:(db + 1) * P],
                                     aT[:, 2 * k:2 * k + 2, t2 * NT:(t2 + 1) * NT],
                                     start=(k == 0), stop=(k == KP - 1), perf_mode=PM)
                nc.vector.tensor_copy(qT[:, db:db + 1, t2 * NT:(t2 + 1) * NT], pq[:])
        pps_cm.__exit__(None, None, None)
        stg_cm.__exit__(None, None, None)
        aT_cm.__exit__(None, None, None)
        wA_cm.__exit__(None, None, None)

        vB_cm = tc.tile_pool(name="vBp", bufs=1)
        vBp = vB_cm.__enter__()
        vB = vBp.tile([P, TJ, C], F8, tag="vB", name="vB")            # 32KB/part
        vg3 = vg_d[:, :].rearrange("(g p) f -> p g f", p=P)
        for g in range(8):
            nc.sync.dma_start(vB[:, 4 * g:4 * (g + 1), :], vg3[:, 4 * g:4 * (g + 1), :])


        # residual chunk load straight into hB (overlaps attention)
        nc.gpsimd.dma_start(hB[:, :, :], dr3(xq[:, :]))

        # ---------------- attention ----------------
        # scores scale: q,k both carry SC -> exp scale = DH^-0.5 / SC^2
        ESC = float(DH) ** -0.5 / (SC * SC)
        et_cm = tc.tile_pool(name="etp", bufs=1)
        etp = et_cm.__enter__()
        pa_cm = tc.tile_pool(name="pa", bufs=1, space="PSUM")
        pa = pa_cm.__enter__()
        for ti in range(TQT):
            for h in range(H):
                po0 = pa.tile([P, NT], F32, tag="po0", bufs=1, name="po0")
                po1 = pa.tile([P, NT], F32, tag="po1", bufs=1, name="po1")
                pr = pa.tile([P, NT], F32, tag="pr", bufs=1, name="pr")
                for tp in range(TJ // 2):
                    psc = pa.tile([P, 2 * NT], F32, tag="s", bufs=2, name="psc")
                    for j in range(2):
                        tj = 2 * tp + j
                        nc.tensor.matmul(psc[:, j * NT:(j + 1) * NT],
                                         kT[:, 2 * h:2 * h + 2, tj * P:(tj + 1) * P],
                                         qT[:, 2 * h:2 * h + 2, ti * NT:(ti + 1) * NT],
                                         perf_mode=PM, skip_group_check=True)
                    et = etp.tile([P, 2, NT], F8, tag="et", bufs=16, name="et")
                    nc.scalar.activation(et[:, :, :], psc[:, :], AF.Exp, scale=ESC)
                    st_, sp_ = (tp == 0), (tp == TJ // 2 - 1)
                    nc.tensor.matmul(po0[:],
                                     vB[:, 2 * tp:2 * tp + 2, h * DH: h * DH + P],
                                     et[:, :, :], start=st_, stop=sp_,
                                     perf_mode=PM, skip_group_check=True)
                    nc.tensor.matmul(po1[:],
                                     vB[:, 2 * tp:2 * tp + 2, h * DH + P:(h + 1) * DH],
                                     et[:, :, :], start=st_, stop=sp_,
                                     perf_mode=PM, skip_group_check=True)
                    nc.tensor.matmul(pr[:], ones8[:, :, :], et[:, :, :],
                                     start=st_, stop=sp_,
                                     perf_mode=PM, skip_group_check=True)
                rec = etp.tile([P, NT], F32, tag="rec", bufs=2, name="rec")
                nc.vector.reciprocal(rec[:], pr[:])
                nc.vector.tensor_mul(oT[:, 2 * h:2 * h + 1, ti * NT:(ti + 1) * NT],
                                     po0[:], rec[:])
                nc.vector.tensor_mul(oT[:, 2 * h + 1:2 * h + 2, ti * NT:(ti + 1) * NT],
                                     po1[:], rec[:])
        pa_cm.__exit__(None, None, None)
        et_cm.__exit__(None, None, None)
        vB_cm.__exit__(None, None, None)
        kT_cm.__exit__(None, None, None)

        # ---------------- Wo + residual ----------------
        pd_cm = tc.tile_pool(name="pd", bufs=1, space="PSUM")
        pd = pd_cm.__enter__()
        for cb in range(CT):
            for t2 in range(TQT):
                ph = pd.tile([P, NT], F32, tag="ph", bufs=4, name="ph")
                for k in range(KP):
                    nc.tensor.matmul(ph[:],
                                     wo_sb[:, 2 * k:2 * k + 2, cb * P:(cb + 1) * P],
                                     oT[:, 2 * k:2 * k + 2, t2 * NT:(t2 + 1) * NT],
                                     start=(k == 0), stop=(k == KP - 1), perf_mode=PM)
                nc.vector.scalar_tensor_tensor(
                    hB[:, cb:cb + 1, t2 * NT:(t2 + 1) * NT],
                    ph[:], 1.0 / (SC * SC),
                    hB[:, cb:cb + 1, t2 * NT:(t2 + 1) * NT], MUL, ADD)
        qo_cm.__exit__(None, None, None)

        # ---------------- FFN ----------------
        for t2 in range(TQT):
            ss = pd.tile([P, NT], F32, tag="ph", bufs=4, name="ss2")
            for ci in range(CT):
                sq = pep.tile([P, NT], BF16, tag="sq2", bufs=2, name="sq2")
                nc.gpsimd.tensor_mul(sq[:], hB[:, ci:ci + 1, t2 * NT:(t2 + 1) * NT],
                                     hB[:, ci:ci + 1, t2 * NT:(t2 + 1) * NT])
                nc.tensor.matmul(ss[:], ones_t[:], sq[:], start=(ci == 0), stop=(ci == CT - 1))
            sqt = pep.tile([P, NT], F32, tag="sqt2", bufs=2, name="sqt2")
            nc.scalar.activation(sqt[:], ss[:], AF.Sqrt, scale=1.0 / C, bias=eps_t[:])
            rn = pep.tile([P, NT], F32, tag="rn2", bufs=2, name="rn2")
            nc.vector.reciprocal(rn[:], sqt[:])
            for ci in range(CT):
                nc.gpsimd.tensor_mul(fB[:, ci:ci + 1, t2 * NT:(t2 + 1) * NT],
                                     hB[:, ci:ci + 1, t2 * NT:(t2 + 1) * NT], rn[:])
        for fb in range(FFB):
            for t2 in range(TQT):
                pu = pd.tile([P, NT], F32, tag="ph", bufs=4, name="pu")
                for k in range(KP):
                    nc.tensor.matmul(pu[:],
                                     w1_sb[:, 2 * k:2 * k + 2, fb * P:(fb + 1) * P],
                                     fB[:, 2 * k:2 * k + 2, t2 * NT:(t2 + 1) * NT],
                                     start=(k == 0), stop=(k == KP - 1), perf_mode=PM)
                nc.scalar.activation(gB[:, fb:fb + 1, t2 * NT:(t2 + 1) * NT],
                                     pu[:], AF.Gelu, scale=1.0 / SC)
        for cb in range(CT):
            for t2 in range(TQT):
                py = pd.tile([P, NT], F32, tag="ph", bufs=4, name="py")
                for k in range(FKP):
                    nc.tensor.matmul(py[:],
                                     w2_sb[:, 2 * k:2 * k + 2, cb * P:(cb + 1) * P],
                                     gB[:, 2 * k:2 * k + 2, t2 * NT:(t2 + 1) * NT],
                                     start=(k == 0), stop=(k == FKP - 1), perf_mode=PM)
                yt = pep.tile([P, NT], F32, tag="yt", bufs=3, name="yt")
                nc.vector.scalar_tensor_tensor(
                    yt[:], py[:], 1.0 / SC,
                    hB[:, cb:cb + 1, t2 * NT:(t2 + 1) * NT], MUL, ADD)
                nc.sync.dma_start(out[cb * P:(cb + 1) * P, t2 * NT:(t2 + 1) * NT], yt[:])
        pd_cm.__exit__(None, None, None)
        pe_cm.__exit__(None, None, None)
        hx_cm.__exit__(None, None, None)
        wB_cm.__exit__(None, None, None)
        dram_cm.__exit__(None, None, None)
        cp_cm.__exit__(None, None, None)

        sched_state, snap = tc.schedule_and_allocate()
        _CACHE["predicted_ns"] = snap.time if snap is not None else None
        try:
            _CACHE["dispatch_ns"] = sched_state.get_inst_dispatch_ns()
        except Exception:
            _CACHE["dispatch_ns"] = None

    nc.finalize()
    return nc


def get_nc():
    if "nc" not in _CACHE:
        _CACHE["nc"] = _build()
    return _CACHE["nc"]


def _prep_inputs(inputs):
    f8 = ml_dtypes.float8_e4m3
    bf = ml_dtypes.bfloat16
    x = np.asarray(inputs["x"], dtype=np.float32)
    g_attn = np.asarray(inputs["g_attn"], dtype=np.float32)
    g_ff = np.asarray(inputs["g_ff"], dtype=np.float32)
    wq8 = (g_attn[:, None] * np.asarray(inputs["Wq"], np.float32) * SC).astype(f8)
    wk8 = (g_attn[:, None] * np.asarray(inputs["Wk"], np.float32) * SC).astype(f8)
    wv8 = (g_attn[:, None] * np.asarray(inputs["Wv"], np.float32) * SC).astype(f8)
    wo8 = (np.asarray(inputs["Wo"], np.float32) * SC).astype(f8)
    w18 = (g_ff[:, None] * np.asarray(inputs["W1"], np.float32) * SC).astype(f8)
    w28 = (np.asarray(inputs["W2"], np.float32) * SC).astype(f8)
    xbf = x.astype(bf)
    in_maps = []
    for core in range(8):
        b, cq = divmod(core, 4)
        in_maps.append({
            "xb": np.ascontiguousarray(xbf[b][:, cq * TQ:(cq + 1) * TQ]),
            "xq": np.ascontiguousarray(x[b][:, cq * TQ:(cq + 1) * TQ]),
            "wq": wq8, "wk": wk8, "wv": wv8, "wo": wo8, "w1": w18, "w2": w28,
        })
    return in_maps


def run(inputs, **kwargs):
    nc = get_nc()
    in_maps = _prep_inputs(inputs)
    res = run_bass_kernel_spmd(nc, in_maps, core_ids=list(range(8)), **kwargs)
    out = np.empty((B, C, T), np.float32)
    for core in range(8):
        b, cq = divmod(core, 4)
        out[b][:, cq * TQ:(cq + 1) * TQ] = res.results[core]["out"]
    return out, res


def kernel(**inputs) -> np.ndarray:
    out, _ = run(inputs)
    return out
